# revision 9
# baseline (speedup 1.0000x reference)
"""Trainium2 Bass kernel for DNAShapeNet (4x conv1d+relu+BN -> 2-layer MLP).

Contract: kernel(**inputs) takes the FULL inputs from setup_inputs() and
returns the FULL [128, 8192] float32 output. Internally shards the batch
across 8 NeuronCores (16 samples each), runs a raw-Bass SPMD kernel, and
concatenates the results.

Design notes:
  - Per core: 16 samples processed as 4 groups of 4. Activations live in
    SBUF as [4 samples x 32 ch = 128 partitions, 3 + 8192 + 3 cols] (zeroed
    margins implement conv zero-padding).
  - Each conv layer = K tap-accumulated fp32r matmuls per 512-col tile:
    lhsT = block-diag(w_tap.T x4) [128,128], rhs = shifted slice of the
    input buffer. PSUM accumulates taps; ScalarE evacuates with fused
    relu+bias; inference BatchNorm is folded into weights/bias on host.
  - FC1 (32->16) is a 1x1 conv (block-diag); FC2 (16->1) produces psum
    [4, 512] which VectorE copies to a staging tile, DMA'd out per group.
  - Raw Bass engine programs with hand-computed semaphore thresholds;
    every instruction carries at most ONE semaphore wait (this container's
    walrus rejects more), and every DMA semaphore increment is preceded by
    a wait on the prior value (SWDGE completions are not FIFO-ordered
    across rings).
"""

import numpy as np

import concourse.bass as bass
import concourse.mybir as mybir
from concourse.bass_utils import run_bass_kernel_spmd

F32 = mybir.dt.float32
F32R = mybir.dt.float32r
RELU = mybir.ActivationFunctionType.Relu
IDENT = mybir.ActivationFunctionType.Identity

EPS = 1e-5
KERNELS = [3, 3, 5, 7]
B_FULL, CIN0, S = 128, 4, 8192
N_CORES = 8
B_LOC = B_FULL // N_CORES          # 16 samples per core
G_PER_CORE = B_LOC // 4            # 4 groups of 4 samples
TW = 512                           # tile width (psum bank, fp32 max moving)
NT = S // TW                       # 16 tiles
MAR = 3                            # buffer margin = max conv pad
W = S + 2 * MAR
NBA = 4                            # rotating psum banks for ACT-evacuated groups
NBF = 2                            # psum banks for FC2 (DVE-evacuated)
C = 32                             # conv channels
ACT_OFS = 8                        # margin-zeroing ACT ops precede evacuations

# const block column layout (also packed identically on host)
OFF_LT = [0, 384, 768, 1408]       # lt_l at [*, OFF_LT[l] : +K_l*128]
OFF_F1 = 2304
OFF_F2 = 2432                      # 32 cols (4 real samples + 28 zero)
OFF_BIAS = 2464                    # 5 cols: conv bias 0..3, fc1 bias
OFF_AFF = 2469                    # 8 cols: (s, t) per conv layer
OFF_ZZ = 2477                      # 3 zero cols (margin source)
CW = 2480


def _fold_params(inp):
    """Fold conv bias + inference BN into lhsT/bias; pack the const block."""
    const = np.zeros((128, CW), np.float64)
    fast = []
    cin = CIN0
    for l, k in enumerate(KERNELS):
        w = np.asarray(inp[f"w{l}"], np.float64)        # [32, cin, k]
        b = np.asarray(inp[f"b{l}"], np.float64)        # [32]
        g = np.asarray(inp[f"g{l}"], np.float64)
        bb = np.asarray(inp[f"bb{l}"], np.float64)
        rm = np.asarray(inp[f"rm{l}"], np.float64)
        rv = np.asarray(inp[f"rv{l}"], np.float64)
        s = g / np.sqrt(rv + EPS)
        t = bb - rm * s
        is_fast = bool(np.all(s > 0) and np.all(t == 0.0))
        fast.append(is_fast)
        if is_fast:
            w_eff = w * s[:, None, None]
            bias = s * b
            aff_s, aff_t = np.ones(C), np.zeros(C)
        else:
            w_eff = w
            bias = b
            aff_s, aff_t = s, t
        for smp in range(4):
            for kk in range(k):
                const[smp * cin:(smp + 1) * cin,
                      OFF_LT[l] + kk * 128 + smp * C:OFF_LT[l] + kk * 128 + (smp + 1) * C] = w_eff[:, :, kk].T
        const[:, OFF_BIAS + l] = np.tile(bias, 4)
        const[:, OFF_AFF + 2 * l] = np.tile(aff_s, 4)
        const[:, OFF_AFF + 2 * l + 1] = np.tile(aff_t, 4)
        cin = C

    fw1 = np.asarray(inp["fw1"], np.float64)            # [16, 32]
    fb1 = np.asarray(inp["fb1"], np.float64)
    fw2 = np.asarray(inp["fw2"], np.float64)            # [1, 16]
    for smp in range(4):
        const[smp * C:(smp + 1) * C, OFF_F1 + smp * C:OFF_F1 + smp * C + 16] = fw1.T
        const[smp * C:smp * C + 16, OFF_BIAS + 4] = 0.0  # placeholder, set below
        const[smp * C:smp * C + 16, OFF_F2 + smp] = fw2[0]  # cols 4..31 stay zero
    bf1 = np.zeros(128)
    for smp in range(4):
        bf1[smp * C:smp * C + 16] = fb1
    const[:, OFF_BIAS + 4] = bf1
    return {"constb": const.astype(np.float32)}, fast


def _build_program(fast):
    nc = bass.Bass()

    x_h = nc.declare_dram_parameter("x", [B_LOC, CIN0, S], F32, isOutput=False)
    const_h = nc.declare_dram_parameter("constb", [128, CW], F32, isOutput=False)
    out_h = nc.declare_dram_parameter("out", [B_LOC, S], F32, isOutput=True)

    # ---------- static schedule bookkeeping ----------
    pe_order = []
    for g in range(G_PER_CORE):
        for l in range(4):
            for t in range(NT):
                pe_order.append(("L", g, l, t))
        for t in range(NT):
            pe_order.append(("F1", g, t))
            pe_order.append(("F2", g, t))
    pe_after = {k: i + 1 for i, k in enumerate(pe_order)}

    act_order = [k for k in pe_order if k[0] in ("L", "F1")]
    act_idx = {k: i for i, k in enumerate(act_order)}
    act_after = {k: ACT_OFS + i + 1 for i, k in enumerate(act_order)}

    # FC2 outputs are packed 4 tiles per psum bank; DVE evacuates per pack.
    n_packs = G_PER_CORE * NT // 4

    from contextlib import ExitStack
    with ExitStack() as st:
        ec = st.enter_context
        Ab = ec(nc.sbuf_tensor("Ab", [128, W], F32R))
        Bb = ec(nc.sbuf_tensor("Bb", [128, W], F32R))
        X0a = ec(nc.sbuf_tensor("X0a", [4 * CIN0, W], F32R))
        X0b = ec(nc.sbuf_tensor("X0b", [4 * CIN0, W], F32R))
        constb = ec(nc.sbuf_tensor("constsb", [128, CW], F32R))
        h0 = ec(nc.sbuf_tensor("h0", [128, TW], F32R))
        h1 = ec(nc.sbuf_tensor("h1", [128, TW], F32R))
        stg0 = ec(nc.sbuf_tensor("stg0", [4, 2 * TW], F32))
        stg1 = ec(nc.sbuf_tensor("stg1", [4, 2 * TW], F32))
        stg2 = ec(nc.sbuf_tensor("stg2", [4, 2 * TW], F32))
        stg3 = ec(nc.sbuf_tensor("stg3", [4, 2 * TW], F32))
        pb0 = ec(nc.psum_tensor("pb0", [128, TW], F32))
        pb1 = ec(nc.psum_tensor("pb1", [128, TW], F32))
        pb2 = ec(nc.psum_tensor("pb2", [128, TW], F32))
        pb3 = ec(nc.psum_tensor("pb3", [128, TW], F32))
        pf0 = ec(nc.psum_tensor("pf0", [128, TW], F32))
        pf1 = ec(nc.psum_tensor("pf1", [128, TW], F32))
        pf2 = ec(nc.psum_tensor("pf2", [128, TW], F32))
        pf3 = ec(nc.psum_tensor("pf3", [128, TW], F32))
        s_w = ec(nc.semaphore("s_w"))
        s_x0a = ec(nc.semaphore("s_x0a"))
        s_x0b = ec(nc.semaphore("s_x0b"))
        s_out0 = ec(nc.semaphore("s_out0"))
        s_out1 = ec(nc.semaphore("s_out1"))
        s_out2 = ec(nc.semaphore("s_out2"))
        s_out3 = ec(nc.semaphore("s_out3"))
        s_pe = ec(nc.semaphore("s_pe"))
        s_act = ec(nc.semaphore("s_act"))
        s_dve = ec(nc.semaphore("s_dve"))
        block = ec(nc.Block())

        banks = [pb0, pb1, pb2, pb3]
        fbanks = [pf0, pf1, pf2, pf3]
        X0 = [X0a, X0b]
        s_x0 = [s_x0a, s_x0b]
        s_out = [s_out0, s_out1, s_out2, s_out3]
        hh = [h0, h1]
        stg = [stg0, stg1, stg2, stg3]
        conv_buf = [None, Ab, Bb, Ab, Bb]

        def lhsT(l, k):
            base = constb[:4 * CIN0, :] if l == 0 else constb[:, :]
            return base[:, OFF_LT[l] + k * 128:OFF_LT[l] + (k + 1) * 128]

        def bias_ap(col):
            return constb[:, OFF_BIAS + col:OFF_BIAS + col + 1].bitcast(F32)

        def aff_ap(col):
            return constb[:, OFF_AFF + col:OFF_AFF + col + 1].bitcast(F32)

        @block.gpsimd
        def _(eng):
            eng.dma_start(out=constb[:, :], in_=const_h[:, :].bitcast(F32R)).then_inc(s_w, 16)

            def load_x0(g):
                if g >= 2:
                    eng.wait_ge(s_pe, pe_after[("L", g - 2, 0, NT - 1)])
                    eng.wait_ge(s_x0[g % 2], 16)  # chain: prior inc observed
                src = x_h[4 * g:4 * (g + 1), :, :].flatten_outer_dims()
                eng.dma_start(out=X0[g % 2][:, MAR:MAR + S], in_=src.bitcast(F32R)).then_inc(s_x0[g % 2], 16)

            def store_pack(p):
                g, tau = p // 4, 4 * (p % 4)
                col = (p % 2) * TW
                for j in range(4):
                    eng.wait_ge(s_dve, 4 * p + j + 1)
                    if p >= 1:
                        eng.wait_ge(s_out[j], 16 * p)  # chain: prior inc observed
                    eng.dma_start(
                        out=out_h[4 * g:4 * (g + 1), (tau + j) * TW:(tau + j + 1) * TW],
                        in_=stg[j][:4, col:col + TW],
                    ).then_inc(s_out[j], 16)

            load_x0(0)
            load_x0(1)
            load_x0(2)
            for p in range(4):
                store_pack(p)
            load_x0(3)
            for p in range(4, n_packs):
                store_pack(p)
            for i in range(4):
                eng.wait_ge(s_out[i], 16 * n_packs)

        @block.tensor
        def _(eng):
            eng.wait_ge(s_w, 16)
            for g in range(G_PER_CORE):
                for l in range(4):
                    k_taps = KERNELS[l]
                    pad = k_taps // 2
                    src = X0[g % 2] if l == 0 else conv_buf[l]
                    for t in range(NT):
                        key = ("L", g, l, t)
                        aidx = act_idx[key]
                        need = aidx + ACT_OFS - NBA + 1 if aidx >= NBA else ACT_OFS
                        if l > 0:
                            need = max(need, act_after[("L", g, l - 1, min(t + 1, NT - 1))])
                        eng.wait_ge(s_act, need)
                        if l == 0 and t == 0:
                            eng.wait_ge(s_x0[g % 2], 16 * (1 + g // 2))
                        bank = banks[aidx % NBA]
                        for k in range(k_taps):
                            lo = t * TW + k - pad + MAR
                            nc.tensor.matmul(
                                bank[:, :],
                                lhsT(l, k),
                                src[:, lo:lo + TW],
                                start=(k == 0), stop=(k == k_taps - 1),
                            ).then_inc(s_pe, 1 if k == k_taps - 1 else 0)
                for t in range(NT):
                    key = ("F1", g, t)
                    aidx = act_idx[key]
                    need = max(aidx + ACT_OFS - NBA + 1, act_after[("L", g, 3, t)])
                    eng.wait_ge(s_act, need)
                    bank = banks[aidx % NBA]
                    nc.tensor.matmul(
                        bank[:, :],
                        constb[:, OFF_F1:OFF_F1 + 128],
                        Bb[:, t * TW + MAR:t * TW + MAR + TW],
                        start=True, stop=True,
                    ).then_inc(s_pe, 1)
                    p = g * 4 + t // 4
                    j = t % 4
                    eng.wait_ge(s_act, act_after[key])
                    if p >= 1:
                        # fbanks[j] freed once previous pack's DVE copy j ran
                        eng.wait_ge(s_dve, 4 * (p - 1) + j + 1)
                    nc.tensor.matmul(
                        fbanks[j][:32, :],
                        constb[:, OFF_F2:OFF_F2 + 32],
                        hh[t % 2][:, :],
                        start=True, stop=True,
                    ).then_inc(s_pe, 1)

        @block.scalar
        def _(eng):
            eng.wait_ge(s_w, 16)
            zsrc = constb[:, OFF_ZZ:OFF_ZZ + MAR]
            for buf in (Ab, Bb):
                nc.scalar.copy(buf[:, 0:MAR], zsrc).then_inc(s_act, 1)
                nc.scalar.copy(buf[:, W - MAR:W], zsrc).then_inc(s_act, 1)
            for buf in (X0a, X0b):
                nc.scalar.copy(buf[:, 0:MAR], zsrc[:4 * CIN0, :]).then_inc(s_act, 1)
                nc.scalar.copy(buf[:, W - MAR:W], zsrc[:4 * CIN0, :]).then_inc(s_act, 1)
            for key in act_order:
                eng.wait_ge(s_pe, pe_after[key])
                aidx = act_idx[key]
                bank = banks[aidx % NBA]
                if key[0] == "L":
                    _, g, l, t = key
                    dst = conv_buf[l + 1][:, t * TW + MAR:t * TW + MAR + TW]
                    if fast[l]:
                        nc.scalar.activation(
                            dst, bank[:, :], RELU, bias=bias_ap(l), scale=1.0,
                        ).then_inc(s_act, 1)
                    else:
                        nc.scalar.activation(
                            bank[:, :], bank[:, :], RELU, bias=bias_ap(l), scale=1.0,
                        )
                        nc.scalar.activation(
                            dst, bank[:, :], IDENT,
                            bias=aff_ap(2 * l + 1), scale=aff_ap(2 * l),
                        ).then_inc(s_act, 1)
                else:
                    _, g, t = key
                    nc.scalar.activation(
                        hh[t % 2][:, :], bank[:, :], RELU, bias=bias_ap(4), scale=1.0,
                    ).then_inc(s_act, 1)

        @block.vector
        def _(eng):
            for p in range(n_packs):
                g, tau = p // 4, 4 * (p % 4)
                col = (p % 2) * TW
                for j in range(4):
                    eng.wait_ge(s_pe, pe_after[("F2", g, tau + j)])
                    if p >= 2:
                        eng.wait_ge(s_out[j], 16 * (p - 1))
                    nc.vector.tensor_copy(
                        stg[j][:4, col:col + TW], fbanks[j][:4, :],
                    ).then_inc(s_dve, 1)

    return nc


def _run(inputs, trace=False):
    params, fast = _fold_params(inputs)
    nc = _build_program(fast)
    x = np.ascontiguousarray(np.asarray(inputs["x"], np.float32))
    in_maps = []
    for c in range(N_CORES):
        m = dict(params)
        m["x"] = np.ascontiguousarray(x[c * B_LOC:(c + 1) * B_LOC])
        in_maps.append(m)
    res = run_bass_kernel_spmd(nc, in_maps, core_ids=list(range(N_CORES)), trace=trace)
    out = np.concatenate([res.results[c]["out"] for c in range(N_CORES)], axis=0)
    fb2 = np.asarray(inputs["fb2"], np.float32)
    if np.any(fb2 != 0):
        out = out + fb2[0]
    return out.astype(np.float32), res


def kernel(**inputs):
    out, _ = _run(inputs, trace=False)
    return out


# revision 31
# speedup vs baseline: 314.8513x; 314.8513x over previous
"""Trainium2 Bass kernel for DNAShapeNet (4x conv1d+relu+BN -> 2-layer MLP).

Contract: kernel(**inputs) takes the FULL inputs from setup_inputs() and
returns the FULL [128, 8192] float32 output. Internally shards the batch
across 8 NeuronCores (16 samples each), runs a raw-Bass SPMD kernel, and
concatenates the results.

Design notes:
  - Per core: 16 samples processed as 4 groups of 4. Activations live in
    SBUF as [4 samples x 32 ch = 128 partitions, 3 + 8192 + 3 cols] (zeroed
    margins implement conv zero-padding).
  - Each conv layer = K tap-accumulated fp32r matmuls per 512-col tile:
    lhsT = block-diag(w_tap.T x4) [128,128], rhs = shifted slice of the
    input buffer. PSUM accumulates taps; ScalarE evacuates with fused
    relu+bias; inference BatchNorm is folded into weights/bias on host.
  - FC1 (32->16) is a 1x1 conv (block-diag); FC2 (16->1) produces psum
    [4, 512] which VectorE copies to a staging tile, DMA'd out per group.
  - Raw Bass engine programs with hand-computed semaphore thresholds;
    every instruction carries at most ONE semaphore wait (this container's
    walrus rejects more), and every DMA semaphore increment is preceded by
    a wait on the prior value (SWDGE completions are not FIFO-ordered
    across rings).
"""

import numpy as np

import concourse.bass as bass
import concourse.mybir as mybir
from concourse.bass_utils import run_bass_kernel_spmd

F32 = mybir.dt.float32
F32R = mybir.dt.float32r
RELU = mybir.ActivationFunctionType.Relu
IDENT = mybir.ActivationFunctionType.Identity

EPS = 1e-5
KERNELS = [3, 3, 5, 7]
B_FULL, CIN0, S = 128, 4, 8192
N_CORES = 8
B_LOC = B_FULL // N_CORES          # 16 samples per core
G_PER_CORE = B_LOC // 4            # 4 groups of 4 samples
TW = 512                           # tile width (psum bank, fp32 max moving)
NT = S // TW                       # 16 tiles
MAR = 3                            # buffer margin = max conv pad
W = S + 2 * MAR
NBA = 6                            # rotating psum banks for ACT-evacuated groups
NBF = 2                            # psum banks for FC2 (DVE-evacuated)
C = 32                             # conv channels
ACT_OFS = 12                       # margin-zeroing ACT ops precede evacuations

# const block column layout (also packed identically on host).
# The first CONSTA cols are the L0-critical prefix, DMA'd first.
OFF_LT = [0, 404, 788, 1428]       # lt_l at [*, OFF_LT[l] : +K_l*128]
OFF_BIAS = 384                     # 5 cols: conv bias 0..3, fc1 bias
OFF_AFF = 389                      # 8 cols: (s, t) per conv layer
OFF_ZZ = 397                       # MAR+1 zero cols (margin source)
CONSTA = 404
CONSTB1 = 788
OFF_F1 = 2324
OFF_F2 = 2452                      # 32 cols (4 real samples + 28 zero)
CW = 2484


def _fold_params(inp):
    """Fold conv bias + inference BN into lhsT/bias; pack the const block."""
    const = np.zeros((128, CW), np.float64)
    fast = []
    cin = CIN0
    for l, k in enumerate(KERNELS):
        w = np.asarray(inp[f"w{l}"], np.float64)        # [32, cin, k]
        b = np.asarray(inp[f"b{l}"], np.float64)        # [32]
        g = np.asarray(inp[f"g{l}"], np.float64)
        bb = np.asarray(inp[f"bb{l}"], np.float64)
        rm = np.asarray(inp[f"rm{l}"], np.float64)
        rv = np.asarray(inp[f"rv{l}"], np.float64)
        s = g / np.sqrt(rv + EPS)
        t = bb - rm * s
        is_fast = bool(np.all(s > 0) and np.all(t == 0.0))
        fast.append(is_fast)
        if is_fast:
            w_eff = w * s[:, None, None]
            bias = s * b
            aff_s, aff_t = np.ones(C), np.zeros(C)
        else:
            w_eff = w
            bias = b
            aff_s, aff_t = s, t
        if l == 0:
            # shift-loaded input: row (r, smp, c) = 16r + 4smp + c; tap r
            for kk in range(k):
                for smp in range(4):
                    const[16 * kk + smp * cin:16 * kk + (smp + 1) * cin,
                          OFF_LT[0] + smp * C:OFF_LT[0] + (smp + 1) * C] = w_eff[:, :, kk].T
        else:
            for smp in range(4):
                for kk in range(k):
                    const[smp * cin:(smp + 1) * cin,
                          OFF_LT[l] + kk * 128 + smp * C:OFF_LT[l] + kk * 128 + (smp + 1) * C] = w_eff[:, :, kk].T
        const[:, OFF_BIAS + l] = np.tile(bias, 4)
        const[:, OFF_AFF + 2 * l] = np.tile(aff_s, 4)
        const[:, OFF_AFF + 2 * l + 1] = np.tile(aff_t, 4)
        cin = C

    fw1 = np.asarray(inp["fw1"], np.float64)            # [16, 32]
    fb1 = np.asarray(inp["fb1"], np.float64)
    fw2 = np.asarray(inp["fw2"], np.float64)            # [1, 16]
    for smp in range(4):
        const[smp * C:(smp + 1) * C, OFF_F1 + smp * C:OFF_F1 + smp * C + 16] = fw1.T
        const[smp * C:smp * C + 16, OFF_BIAS + 4] = 0.0  # placeholder, set below
        const[smp * C:smp * C + 16, OFF_F2 + smp] = fw2[0]  # cols 4..31 stay zero
    bf1 = np.zeros(128)
    for smp in range(4):
        bf1[smp * C:smp * C + 16] = fb1
    const[:, OFF_BIAS + 4] = bf1
    return {"constb": const.astype(np.float32)}, fast


def _build_program(fast, g_loop=G_PER_CORE):
    # g_loop > G_PER_CORE repeats the whole computation (for steady-state
    # timing): group g processes batch slice (g % G_PER_CORE).
    nc = bass.Bass()

    x_h = nc.declare_dram_parameter("x", [B_LOC, CIN0, S], F32, isOutput=False)
    const_h = nc.declare_dram_parameter("constb", [128, CW], F32, isOutput=False)
    out_h = nc.declare_dram_parameter("out", [B_LOC, S], F32, isOutput=True)

    # ---------- static schedule bookkeeping ----------
    pe_order = []
    for g in range(g_loop):
        for l in range(4):
            for t in range(NT):
                pe_order.append(("L", g, l, t))
        pe_order.append(("F1", g, 0))
        pe_order.append(("F1", g, 1))
        for t in range(NT):
            if t + 2 < NT:
                pe_order.append(("F1", g, t + 2))
            pe_order.append(("F2", g, t))
    pe_after = {k: i + 1 for i, k in enumerate(pe_order)}

    act_order = [k for k in pe_order if k[0] in ("L", "F1")]
    act_idx = {k: i for i, k in enumerate(act_order)}
    act_after = {k: ACT_OFS + i + 1 for i, k in enumerate(act_order)}

    # FC2 outputs are packed 4 tiles per psum bank; DVE evacuates per pack.
    n_packs = g_loop * NT // 4

    from contextlib import ExitStack
    with ExitStack() as st:
        ec = st.enter_context
        Ab = ec(nc.sbuf_tensor("Ab", [128, W], F32R))
        Bb = ec(nc.sbuf_tensor("Bb", [128, W], F32R))
        X0a = ec(nc.sbuf_tensor("X0a", [48, W], F32R))
        X0b = ec(nc.sbuf_tensor("X0b", [48, W], F32R))
        constb = ec(nc.sbuf_tensor("constsb", [128, CW], F32R))
        h0 = ec(nc.sbuf_tensor("h0", [128, TW], F32R))
        h1 = ec(nc.sbuf_tensor("h1", [128, TW], F32R))
        h2 = ec(nc.sbuf_tensor("h2", [128, TW], F32R))
        h3 = ec(nc.sbuf_tensor("h3", [128, TW], F32R))
        stgb = ec(nc.sbuf_tensor("stgb", [4, S], F32))
        pb0 = ec(nc.psum_tensor("pb0", [128, TW], F32))
        pb1 = ec(nc.psum_tensor("pb1", [128, TW], F32))
        pb2 = ec(nc.psum_tensor("pb2", [128, TW], F32))
        pb3 = ec(nc.psum_tensor("pb3", [128, TW], F32))
        pb4 = ec(nc.psum_tensor("pb4", [128, TW], F32))
        pb5 = ec(nc.psum_tensor("pb5", [128, TW], F32))
        pf0 = ec(nc.psum_tensor("pf0", [128, TW], F32))
        pf1 = ec(nc.psum_tensor("pf1", [128, TW], F32))
        s_w = ec(nc.semaphore("s_w"))
        s_x0 = [[ec(nc.semaphore(f"s_x0_{p}_{r}")) for r in range(3)] for p in range(2)]
        s_out = ec(nc.semaphore("s_out"))
        s_pe = ec(nc.semaphore("s_pe"))
        s_act = ec(nc.semaphore("s_act"))
        s_dve = ec(nc.semaphore("s_dve"))
        s_gp = ec(nc.semaphore("s_gp"))
        block = ec(nc.Block())

        banks = [pb0, pb1, pb2, pb3, pb4, pb5]
        fbanks = [pf0, pf1]
        X0 = [X0a, X0b]
        hh = [h0, h1, h2, h3]
        conv_buf = [None, Ab, Bb, Ab, Bb]

        def lhsT(l, k):
            if l == 0:
                return constb[:48, OFF_LT[0]:OFF_LT[0] + 128]
            return constb[:, OFF_LT[l] + k * 128:OFF_LT[l] + (k + 1) * 128]

        def bias_ap(col):
            return constb[:, OFF_BIAS + col:OFF_BIAS + col + 1].bitcast(F32)

        def aff_ap(col):
            return constb[:, OFF_AFF + col:OFF_AFF + col + 1].bitcast(F32)

        QCH = 4                       # x0 load chunks per shift
        CHW = S // QCH

        def x0_dma(eng, g, r, chunk):
            # chunk c covers dst cols [d_lo + c*CHW, ...): tiles of that quarter
            gi = g % G_PER_CORE
            xf = x_h[4 * gi:4 * (gi + 1), :, :].flatten_outer_dims()  # [16, S]
            base = 16 * QCH * (g // 2) + 16 * chunk
            s_lo = max(0, r - 1)
            d_lo = MAR + max(0, 1 - r)
            n = min(S, S + r - 1) - s_lo
            c_lo = chunk * CHW
            c_n = min(CHW, n - c_lo)
            if base > 0:
                eng.wait_ge(s_x0[g % 2][r], base)
            eng.dma_start(
                out=X0[g % 2][16 * r:16 * r + 16, d_lo + c_lo:d_lo + c_lo + c_n],
                in_=xf[:, s_lo + c_lo:s_lo + c_lo + c_n].bitcast(F32R),
            ).then_inc(s_x0[g % 2][r], 16)

        @block.sync
        def _(eng):
            NST = 8
            STW = S // NST

            def store_quarter(g, q):
                gi = g % G_PER_CORE
                eng.wait_ge(s_dve, 16 * g + 2 * (q + 1))
                prior = 16 * NST * g + 16 * q
                if prior > 0:
                    eng.wait_ge(s_out, prior)  # chain: prior inc observed
                eng.dma_start(
                    out=out_h[4 * gi:4 * (gi + 1), q * STW:(q + 1) * STW],
                    in_=stgb[:4, q * STW:(q + 1) * STW],
                ).then_inc(s_out, 16)

            eng.dma_start(out=constb[:, :CONSTA],
                          in_=const_h[:, :CONSTA].bitcast(F32R)).then_inc(s_w, 16)
            for cch in range(QCH):
                x0_dma(eng, 0, 1, cch)
            eng.wait_ge(s_w, 16)
            eng.dma_start(out=constb[:, CONSTA:CONSTB1],
                          in_=const_h[:, CONSTA:CONSTB1].bitcast(F32R)).then_inc(s_w, 16)
            eng.wait_ge(s_w, 32)
            eng.dma_start(out=constb[:, CONSTB1:],
                          in_=const_h[:, CONSTB1:].bitcast(F32R)).then_inc(s_w, 16)
            for r in (1, 2):
                for cch in range(QCH):
                    x0_dma(eng, 1, r, cch)
            for g in range(2, g_loop):
                eng.wait_ge(s_pe, pe_after[("L", g - 2, 0, NT - 1)])
                for r in (1, 2):
                    for cch in range(QCH):
                        x0_dma(eng, g, r, cch)
                for q in range(NST):
                    store_quarter(g - 2, q)
            for g in (g_loop - 2, g_loop - 1):
                for q in range(NST):
                    store_quarter(g, q)
            eng.wait_ge(s_out, 16 * NST * g_loop)

        @block.gpsimd
        def _(eng):
            def load_x0(g):
                if g >= 2:
                    eng.wait_ge(s_pe, pe_after[("L", g - 2, 0, NT - 1)])
                for cch in range(QCH):
                    x0_dma(eng, g, 0, cch)

            for g in range(g_loop):
                load_x0(g)

        @block.tensor
        def _(eng):
            eng.wait_ge(s_w, 16)
            for g in range(g_loop):
                for l in range(4):
                    k_taps = KERNELS[l]
                    pad = k_taps // 2
                    src = X0[g % 2] if l == 0 else conv_buf[l]
                    for t in range(NT):
                        key = ("L", g, l, t)
                        aidx = act_idx[key]
                        need = aidx + ACT_OFS - NBA + 1 if aidx >= NBA else ACT_OFS
                        if l > 0:
                            need = max(need, act_after[("L", g, l - 1, min(t + 1, NT - 1))])
                        eng.wait_ge(s_act, need)
                        if g == 0 and l == 1 and t == 0:
                            eng.wait_ge(s_w, 32)
                        if g == 0 and l == 2 and t == 0:
                            eng.wait_ge(s_w, 48)
                        if l == 0 and t % (NT // 4) == 0:
                            cch = t // (NT // 4)
                            for r in range(3):
                                eng.wait_ge(s_x0[g % 2][r], 64 * (g // 2) + 16 * (cch + 1))
                        bank = banks[aidx % NBA]
                        if l == 0:
                            nc.tensor.matmul(
                                bank[:, :],
                                lhsT(0, 0),
                                src[:48, t * TW + MAR:t * TW + MAR + TW],
                                start=True, stop=True,
                            ).then_inc(s_pe, 1)
                        else:
                            for k in range(k_taps):
                                lo = t * TW + k - pad + MAR
                                nc.tensor.matmul(
                                    bank[:, :],
                                    lhsT(l, k),
                                    src[:, lo:lo + TW],
                                    start=(k == 0), stop=(k == k_taps - 1),
                                ).then_inc(s_pe, 1 if k == k_taps - 1 else 0)
                def emit_f1(t):
                    key = ("F1", g, t)
                    aidx = act_idx[key]
                    need = max(aidx + ACT_OFS - NBA + 1, act_after[("L", g, 3, t)])
                    eng.wait_ge(s_act, need)
                    nc.tensor.matmul(
                        banks[aidx % NBA][:, :],
                        constb[:, OFF_F1:OFF_F1 + 128],
                        Bb[:, t * TW + MAR:t * TW + MAR + TW],
                        start=True, stop=True,
                    ).then_inc(s_pe, 1)

                def emit_f2(t):
                    p = g * 4 + t // 4
                    j = t % 4
                    eng.wait_ge(s_act, act_after[("F1", g, t)])
                    # fbanks[j % 2] freed once the copy 2 tile-slots back ran
                    prev = 4 * p + j - 2 if j >= 2 else (4 * (p - 1) + j + 2 if p >= 1 else -1)
                    if prev >= 0:
                        eng.wait_ge(s_dve, prev + 1)
                    nc.tensor.matmul(
                        fbanks[j % 2][:32, :],
                        constb[:, OFF_F2:OFF_F2 + 32],
                        hh[t % 4][:, :],
                        start=True, stop=True,
                    ).then_inc(s_pe, 1)

                emit_f1(0)
                emit_f1(1)
                for t in range(NT):
                    if t + 2 < NT:
                        emit_f1(t + 2)
                    emit_f2(t)

        @block.scalar
        def _(eng):
            gp_count = [0]
            x0_dma(eng, 0, 2, 0)
            x0_dma(eng, 0, 2, 1)
            eng.wait_ge(s_w, 16)
            zsrc = constb[:, OFF_ZZ:OFF_ZZ + MAR]
            for buf in (Ab, Bb):
                nc.scalar.copy(buf[:, 0:MAR], zsrc).then_inc(s_act, 1)
                nc.scalar.copy(buf[:, W - MAR:W], zsrc).then_inc(s_act, 1)
            for buf in (X0a, X0b):
                # stale cells never covered by the shifted input DMAs:
                # left: [0,3) all blocks + col 3 on the r=0 block
                # right: [8195,8198) all blocks + col 8194 on the r=2 block
                nc.scalar.copy(buf[:48, 0:MAR], zsrc[:48, :]).then_inc(s_act, 1)
                nc.scalar.copy(buf[:16, MAR:MAR + 1], constb[:16, OFF_ZZ:OFF_ZZ + 1]).then_inc(s_act, 1)
                nc.scalar.copy(buf[:48, W - MAR:W], zsrc[:48, :]).then_inc(s_act, 1)
                nc.scalar.copy(buf[32:48, W - MAR - 1:W - MAR], constb[32:48, OFF_ZZ:OFF_ZZ + 1]).then_inc(s_act, 1)
            x0_dma(eng, 0, 2, 2)
            x0_dma(eng, 0, 2, 3)
            for key in act_order:
                eng.wait_ge(s_pe, pe_after[key])
                aidx = act_idx[key]
                bank = banks[aidx % NBA]
                if key[0] == "L":
                    _, g, l, t = key
                    dst = conv_buf[l + 1][:, t * TW + MAR:t * TW + MAR + TW]
                    if fast[l]:
                        nc.scalar.activation(
                            dst, bank[:, :], RELU, bias=bias_ap(l), scale=1.0,
                        ).then_inc(s_act, 1)
                    else:
                        gp_count[0] += 1
                        nc.scalar.activation(
                            bank[:, :], bank[:, :], RELU, bias=bias_ap(l), scale=1.0,
                        ).then_inc(s_gp, 1)
                        eng.wait_ge(s_gp, gp_count[0])
                        nc.scalar.activation(
                            dst, bank[:, :], IDENT,
                            bias=aff_ap(2 * l + 1), scale=aff_ap(2 * l),
                        ).then_inc(s_act, 1)
                else:
                    _, g, t = key
                    nc.scalar.activation(
                        hh[t % 4][:, :], bank[:, :], RELU, bias=bias_ap(4), scale=1.0,
                    ).then_inc(s_act, 1)

        @block.vector
        def _(eng):
            for p in range(n_packs):
                g, tau = p // 4, 4 * (p % 4)
                for j in range(4):
                    t = tau + j
                    eng.wait_ge(s_pe, pe_after[("F2", g, t)])
                    if t == 0 and g >= 1:
                        eng.wait_ge(s_out, 16 * 8 * g)  # stg reused across groups
                    nc.vector.tensor_copy(
                        stgb[:4, t * TW:(t + 1) * TW], fbanks[j % 2][:4, :],
                    ).then_inc(s_dve, 1)

    return nc


def _run(inputs, trace=False):
    params, fast = _fold_params(inputs)
    nc = _build_program(fast)
    x = np.ascontiguousarray(np.asarray(inputs["x"], np.float32))
    in_maps = []
    for c in range(N_CORES):
        m = dict(params)
        m["x"] = np.ascontiguousarray(x[c * B_LOC:(c + 1) * B_LOC])
        in_maps.append(m)
    res = run_bass_kernel_spmd(nc, in_maps, core_ids=list(range(N_CORES)), trace=trace)
    out = np.concatenate([res.results[c]["out"] for c in range(N_CORES)], axis=0)
    fb2 = np.asarray(inputs["fb2"], np.float32)
    if np.any(fb2 != 0):
        out = out + fb2[0]
    return out.astype(np.float32), res


def kernel(**inputs):
    out, _ = _run(inputs, trace=False)
    return out
